# revision 26
# baseline (speedup 1.0000x reference)
"""Trainium2 Bass kernel for nn_BranchingQNetwork (12-branch dueling Q-MLP).

Strategy: hybrid sharding — 4 branch-groups x 2 batch-halves over 8 cores.
Each core computes 3 branches for 4096 rows. All tensors bf16 (fp32 PSUM
accumulation), so the full 3-branch weight set (~16 MB) is SBUF-resident,
prefetched one branch ahead; matmuls never wait on HBM. The dueling head
(v + a - mean(a)) is linear and folded into a single [512, 12] weight on
the host; head output is action-major [12, batch] on-chip, transposed on
the host.

PE schedule: batch tiles are processed in pairs of 512 columns sharing one
LDWEIGHTS (each stationary feeds two N=512 matmuls), which halves PE
weight-load traffic and doubles the completion-dependency margin. PSUM:
4 "accumulator" banks (L2/L3 m-groups of 2 x 2 halves, k-outer) + 4
"streaming" banks (L1, head). L1 matmuls of the next iteration and the
previous iteration's head are emitted as fillers at the accumulation-group
boundaries to hide PSUM-drain latency.
"""
import sys

sys.path.insert(0, "/opt/trn_rl_repo")

import numpy as np
import ml_dtypes

BF16 = ml_dtypes.bfloat16

# problem dims (hardcoded per harness contract)
B = 8192
OBS = 249
NB = 12
NA = 11
NODE = 45
GRP = 17
D0 = 62
D0P = 128            # L1 contraction zero-padded to full PE height (avoids
                     # row_grp switches that expose LDWEIGHTS on every
                     # L1<->L2/L3 transition)
D1 = 2048
D2 = 1024
D3 = 512

NCORES = 8
NGRP = 4             # branch groups
NBL = NB // NGRP     # 3 branches per core
LBH = B // 2         # 4096 rows per core (batch halves)
BT = 1024            # batch tile (pair of 512-column PSUM halves)
BH = 512             # PSUM half-tile
NBT = LBH // BT      # 4 batch tiles
M1 = D1 // 128       # 16 output tiles of layer 1
K2 = D1 // 128       # 16 contraction tiles of layer 2
M2 = D2 // 128       # 8
K3 = D2 // 128       # 8
M3 = D3 // 128       # 4
KH = D3 // 128       # 4
NAP = 12             # head width padded even

_NC_CACHE = {}
LAST_RESULT = None


def _build_nc():
    if "nc" in _NC_CACHE:
        return _NC_CACHE["nc"]
    from concourse import bacc
    import concourse.mybir as mybir
    import concourse.tile as tile

    f32 = mybir.dt.float32
    bf16 = mybir.dt.bfloat16
    Relu = mybir.ActivationFunctionType.Relu
    ADD = mybir.AluOpType.add
    MAX = mybir.AluOpType.max

    nc = bacc.Bacc("TRN2")

    px_d = nc.declare_dram_parameter("pxp", [NBL, D0P, LBH], bf16, isOutput=False)
    W1_d = nc.declare_dram_parameter("W1p", [NBL, D0P, D1], bf16, isOutput=False)
    W2_d = nc.declare_dram_parameter("W2p", [NBL, 128, K2, D2], bf16, isOutput=False)
    W3_d = nc.declare_dram_parameter("W3p", [NBL, 128, K3, D3], bf16, isOutput=False)
    Wq_d = nc.declare_dram_parameter("Wqp", [NBL, 128, KH, NAP], bf16, isOutput=False)
    b_d = nc.declare_dram_parameter("bp", [NBL, 128, M1 + M2 + M3], f32, isOutput=False)
    bq_d = nc.declare_dram_parameter("bqp", [NBL, NAP, 1], f32, isOutput=False)
    out_d = nc.declare_dram_parameter("out", [NBL, NAP, LBH], f32, isOutput=True)

    with tile.TileContext(nc) as tc:
        with (
            tc.tile_pool(name="wp1", bufs=2) as wp1,
            tc.tile_pool(name="wp2", bufs=2) as wp2,
            tc.tile_pool(name="wp3", bufs=2) as wp3,
            tc.tile_pool(name="wpq", bufs=2) as wpq,
            tc.tile_pool(name="bbp", bufs=2) as bbp,
            tc.tile_pool(name="pxp_s", bufs=2) as pxp,
            tc.tile_pool(name="h1p", bufs=2) as h1p,
            tc.tile_pool(name="actp", bufs=1) as actp,
            tc.tile_pool(name="osp", bufs=2) as osp,
            tc.tile_pool(name="psA", bufs=4, space="PSUM") as psA,
            tc.tile_pool(name="psS", bufs=4, space="PSUM") as psS,
        ):
            h2 = actp.tile([128, K3, BT], bf16, tag="h2")
            h3 = actp.tile([128, KH, BT], bf16, tag="h3")

            iters = [(br, bt) for br in range(NBL) for bt in range(NBT)]
            loaded = {}
            pxs = {}
            h1s = {}

            def load_branch(br):
                first = br == 0
                s_eng = nc.scalar if first else nc.sync
                w1t = wp1.tile([D0P, D1], bf16, tag="w1", name=f"w1_{br}")
                if first:
                    # branch 0 gates pipeline start: chunk W1/W2 on the sync
                    # ring so slices land just ahead of their use. Keep the
                    # scalar queue free — its engine must run the L1 drains.
                    for c in range(4):
                        nc.sync.dma_start(w1t[:, 512 * c:512 * (c + 1)],
                                          W1_d[br][:, 512 * c:512 * (c + 1)])
                else:
                    nc.sync.dma_start(w1t[:], W1_d[br])
                wqt = wpq.tile([128, KH, NAP], bf16, tag="wq", name=f"wq_{br}")
                s_eng.dma_start(wqt[:], Wq_d[br])
                btile = bbp.tile([128, M1 + M2 + M3], f32, tag="b", name=f"b_{br}")
                s_eng.dma_start(btile[:], b_d[br])
                bqt = bbp.tile([NAP, 1], f32, tag="bq", name=f"bq_{br}")
                s_eng.dma_start(bqt[:], bq_d[br])
                w2t = wp2.tile([128, K2, D2], bf16, tag="w2", name=f"w2_{br}")
                nch = 8 if first else 4
                kc = K2 // nch
                for c in range(nch):
                    nc.sync.dma_start(
                        w2t[:, kc * c:kc * (c + 1), :],
                        W2_d[br][:, kc * c:kc * (c + 1), :],
                    )
                w3t = wp3.tile([128, K3, D3], bf16, tag="w3", name=f"w3_{br}")
                nc.sync.dma_start(w3t[:], W3_d[br])
                loaded[br] = (w1t, w2t, w3t, wqt, btile, bqt)

            def load_px(idx):
                br, bt = iters[idx]
                bsl = slice(bt * BT, (bt + 1) * BT)
                px = pxp.tile([D0P, BT], bf16, tag="px", name=f"px_{idx}")
                nc.scalar.dma_start(px[:], px_d[br][:, bsl])
                pxs[idx] = px
                h1s[idx] = h1p.tile([128, M1, BT], bf16, tag="h1", name=f"h1_{idx}")

            def drain_split(dst, ps, bias, j):
                a, b = dst[:, 0:BH // 2], dst[:, BH // 2:BH]
                pa, pb = ps[:, 0:BH // 2], ps[:, BH // 2:BH]
                if j % 2 == 0:
                    nc.scalar.activation(a, pa, Relu, bias=bias, scale=1.0)
                    nc.vector.tensor_scalar(b, pb, bias, 0.0, ADD, MAX)
                else:
                    nc.vector.tensor_scalar(a, pa, bias, 0.0, ADD, MAX)
                    nc.scalar.activation(b, pb, Relu, bias=bias, scale=1.0)

            def emit_L1_pair(idx, m):
                # one stationary, two N=512 matmuls (batch halves)
                br, _ = iters[idx]
                w1t, _, _, _, btile, _ = loaded[br]
                for h in range(2):
                    ps = psS.tile([128, BH], f32, tag="ps",
                                  name=f"l1ps_{idx}_{m}_{h}")
                    mm = nc.tensor.matmul(
                        ps[:], w1t[:, m * 128:(m + 1) * 128],
                        pxs[idx][:, h * BH:(h + 1) * BH],
                        start=True, stop=True,
                    )
                    if h == 1:
                        mm.ins.ldweights = False  # stationary already loaded
                    drain_split(h1s[idx][:, m, h * BH:(h + 1) * BH], ps[:],
                                btile[:, m:m + 1], m + h)

            def head_state(idx):
                psh = [psS.tile([NAP, BH], f32, tag="ps",
                                name=f"psh_{idx}_{h}") for h in range(2)]
                return (idx, psh)

            def head_mms(state, k):
                idx, psh = state
                br, _ = iters[idx]
                wqt = loaded[br][3]
                for h in range(2):
                    mm = nc.tensor.matmul(
                        psh[h][:], wqt[:, k, :],
                        h3[:, k, h * BH:(h + 1) * BH],
                        start=(k == 0), stop=(k == KH - 1),
                    )
                    if h == 1:
                        mm.ins.ldweights = False

            def head_finish(state, out_eng=None):
                idx, psh = state
                br, bt = iters[idx]
                bqt = loaded[br][5]
                ot = osp.tile([NAP, BT], f32, tag="os", name=f"ot_{idx}")
                for h in range(2):
                    nc.vector.tensor_scalar_add(
                        ot[:, h * BH:(h + 1) * BH], psh[h][:], bqt[:])
                bsl = slice(bt * BT, (bt + 1) * BT)
                (out_eng or nc.gpsimd).dma_start(out_d[br][:, bsl], ot[:])

            def emit_head(idx, out_eng=None):
                st = head_state(idx)
                for k in range(KH):
                    head_mms(st, k)
                head_finish(st, out_eng)

            # prologue: px + W1 land first; while they are in flight, run
            # throwaway matmuls on a zeroed tile — they overlap the DMA wait
            # and push the PE HAM clock-gate to full rate (2.4 GHz) before
            # real work arrives
            load_px(0)
            load_branch(0)
            warm = actp.tile([128, BH], bf16, tag="warm")
            nc.vector.memset(warm[:], 0)
            for w in range(10):
                ps = psS.tile([128, BH], f32, tag="ps", name=f"warm_{w}")
                nc.tensor.matmul(ps[:], warm[:, 0:128], warm[:],
                                 start=True, stop=True)
            for m in range(M1):
                emit_L1_pair(0, m)

            for idx, (br, bt) in enumerate(iters):
                w1t, w2t, w3t, wqt, btile, bqt = loaded[br]
                h1 = h1s[idx]
                nxt = idx + 1
                have_nxt = nxt < len(iters)
                if have_nxt:
                    load_px(nxt)
                if bt == 0 and br + 1 < NBL:
                    load_branch(br + 1)

                fill = iter(range(M1))  # next-iteration L1 m indices

                def fillers(n):
                    if have_nxt:
                        for _ in range(n):
                            m = next(fill, None)
                            if m is not None:
                                emit_L1_pair(nxt, m)

                last = idx == len(iters) - 1
                # prev head at iteration top covers the ps3(idx-1) -> L2 g0
                # PSUM drain latency; in the last iteration (which has no L1
                # fillers) it is spread across the L2 group boundaries instead
                hd_prev = None
                if idx > 0:
                    if last:
                        hd_prev = head_state(idx - 1)
                        head_mms(hd_prev, 0)
                    else:
                        emit_head(idx - 1)

                # ---- L2 [2048 -> 1024]: 4 m-groups of 2, k-outer,
                #      each stationary feeds both batch halves ----
                for g in range(4):
                    ps2 = [psA.tile([128, BH], f32, tag="ps",
                                    name=f"ps2_{idx}_{g}_{j}") for j in range(4)]
                    for k in range(K2):
                        for jm in range(2):
                            m = 2 * g + jm
                            for h in range(2):
                                mm = nc.tensor.matmul(
                                    ps2[2 * jm + h][:],
                                    w2t[:, k, m * 128:(m + 1) * 128],
                                    h1[:, k, h * BH:(h + 1) * BH],
                                    start=(k == 0), stop=(k == K2 - 1),
                                )
                                if h == 1:
                                    mm.ins.ldweights = False
                    for jm in range(2):
                        m = 2 * g + jm
                        for h in range(2):
                            drain_split(h2[:, m, h * BH:(h + 1) * BH],
                                        ps2[2 * jm + h][:],
                                        btile[:, M1 + m:M1 + m + 1], jm + h)
                    fillers(2)  # hide this group's drains
                    if hd_prev is not None and g < KH - 1:
                        head_mms(hd_prev, g + 1)
                    if hd_prev is not None and g == KH - 1:
                        head_finish(hd_prev)

                # ---- L3 [1024 -> 512]: 2 m-groups of 2, k-outer ----
                for g in range(2):
                    ps3 = [psA.tile([128, BH], f32, tag="ps",
                                    name=f"ps3_{idx}_{g}_{j}") for j in range(4)]
                    for k in range(K3):
                        for jm in range(2):
                            m = 2 * g + jm
                            for h in range(2):
                                mm = nc.tensor.matmul(
                                    ps3[2 * jm + h][:],
                                    w3t[:, k, m * 128:(m + 1) * 128],
                                    h2[:, k, h * BH:(h + 1) * BH],
                                    start=(k == 0), stop=(k == K3 - 1),
                                )
                                if h == 1:
                                    mm.ins.ldweights = False
                        if k == 3:
                            fillers(2)  # spread remaining L1 work
                    for jm in range(2):
                        m = 2 * g + jm
                        for h in range(2):
                            drain_split(h3[:, m, h * BH:(h + 1) * BH],
                                        ps3[2 * jm + h][:],
                                        btile[:, M1 + M2 + m:M1 + M2 + m + 1],
                                        jm + h)
                    fillers(2)
                    if last:
                        if g == 0:
                            hd_last = head_state(idx)
                            head_mms(hd_last, 0)
                            head_mms(hd_last, 1)
                        else:
                            head_mms(hd_last, 2)
                            head_mms(hd_last, 3)
                            # final out through the scalar HWDGE queue: the
                            # gpsimd SWDGE latency would sit exposed at the
                            # kernel tail
                            head_finish(hd_last, out_eng=nc.scalar)
                fillers(M1)  # flush any remaining next-L1 pairs

    nc.compile()
    _NC_CACHE["nc"] = nc
    return nc


def _pack_weights(W1, b1, W2, b2, W3, b3, Wv, bv, Wa, ba):
    f = np.float32
    W1p = np.zeros((NB, D0P, D1), BF16)                           # [12, 128, 2048]
    W1p[:, :D0, :] = np.ascontiguousarray(W1).astype(BF16)
    W2p = np.ascontiguousarray(
        W2.reshape(NB, K2, 128, D2).transpose(0, 2, 1, 3)).astype(BF16)
    W3p = np.ascontiguousarray(
        W3.reshape(NB, K3, 128, D3).transpose(0, 2, 1, 3)).astype(BF16)
    # fold dueling head: q = h @ (Wv + Wa - mean(Wa)) + (bv + ba - mean(ba))
    Wq = Wv + Wa - Wa.mean(axis=2, keepdims=True)                 # [12, 512, 11]
    bq = bv + ba - ba.mean(axis=1, keepdims=True)                 # [12, 11]
    Wq = np.concatenate([Wq, np.zeros((NB, D3, NAP - NA), Wq.dtype)], axis=2)
    bq = np.concatenate([bq, np.zeros((NB, NAP - NA), bq.dtype)], axis=1)
    Wqp = np.ascontiguousarray(
        Wq.reshape(NB, KH, 128, NAP).transpose(0, 2, 1, 3)).astype(BF16)
    bp = np.concatenate(
        [
            b1.reshape(NB, M1, 128).transpose(0, 2, 1),
            b2.reshape(NB, M2, 128).transpose(0, 2, 1),
            b3.reshape(NB, M3, 128).transpose(0, 2, 1),
        ],
        axis=2,
    ).astype(f)                                                   # [12, 128, 28]
    bqp = np.ascontiguousarray(bq[:, :, None], f)                 # [12, 12, 1]
    return W1p, W2p, W3p, Wqp, bp, bqp


def kernel(x, W1, b1, W2, b2, W3, b3, Wv, bv, Wa, ba):
    global LAST_RESULT
    from concourse.bass_utils import run_bass_kernel_spmd

    x = np.asarray(x, np.float32)
    args = [np.asarray(a, np.float32) for a in (W1, b1, W2, b2, W3, b3, Wv, bv, Wa, ba)]
    W1p, W2p, W3p, Wqp, bp, bqp = _pack_weights(*args)

    node = x[:, :NODE]                                  # [B, 45]
    groups = x[:, NODE:].reshape(B, NB, GRP)            # [B, 12, 17]

    nc = _build_nc()
    in_maps = []
    for c in range(NCORES):
        g, h = c // 2, c % 2
        rows = slice(h * LBH, (h + 1) * LBH)
        brs = slice(g * NBL, (g + 1) * NBL)
        pxc = np.zeros((NBL, D0P, LBH), BF16)
        nT = np.ascontiguousarray(node[rows].T).astype(BF16)
        for j in range(NBL):
            pxc[j, :NODE] = nT
            pxc[j, NODE:D0] = groups[rows, g * NBL + j].T
        in_maps.append({
            "pxp": pxc,
            "W1p": W1p[brs], "W2p": W2p[brs], "W3p": W3p[brs], "Wqp": Wqp[brs],
            "bp": bp[brs], "bqp": bqp[brs],
        })

    res = run_bass_kernel_spmd(nc, in_maps, list(range(NCORES)))
    LAST_RESULT = res

    out = np.empty((NB, B, NA), np.float32)
    for c in range(NCORES):
        g, h = c // 2, c % 2
        rows = slice(h * LBH, (h + 1) * LBH)
        oc = res.results[c]["out"]                      # [3, 12, 4096]
        for j in range(NBL):
            out[g * NBL + j, rows, :] = oc[j, :NA, :].T
    return out


# revision 27
# speedup vs baseline: 1.0034x; 1.0034x over previous
"""Trainium2 Bass kernel for nn_BranchingQNetwork (12-branch dueling Q-MLP).

Strategy: hybrid sharding — 4 branch-groups x 2 batch-halves over 8 cores.
Each core computes 3 branches for 4096 rows. All tensors bf16 (fp32 PSUM
accumulation), so the full 3-branch weight set (~16 MB) is SBUF-resident,
prefetched one branch ahead; matmuls never wait on HBM. The dueling head
(v + a - mean(a)) is linear and folded into a single [512, 12] weight on
the host; head output is action-major [12, batch] on-chip, transposed on
the host.

PE schedule: batch tiles are processed in pairs of 512 columns sharing one
LDWEIGHTS (each stationary feeds two N=512 matmuls), which halves PE
weight-load traffic and doubles the completion-dependency margin. PSUM:
4 "accumulator" banks (L2/L3 m-groups of 2 x 2 halves, k-outer) + 4
"streaming" banks (L1, head). L1 matmuls of the next iteration and the
previous iteration's head are emitted as fillers at the accumulation-group
boundaries to hide PSUM-drain latency.
"""
import sys

sys.path.insert(0, "/opt/trn_rl_repo")

import numpy as np
import ml_dtypes

BF16 = ml_dtypes.bfloat16

# problem dims (hardcoded per harness contract)
B = 8192
OBS = 249
NB = 12
NA = 11
NODE = 45
GRP = 17
D0 = 62
D0P = 128            # L1 contraction zero-padded to full PE height (avoids
                     # row_grp switches that expose LDWEIGHTS on every
                     # L1<->L2/L3 transition)
D1 = 2048
D2 = 1024
D3 = 512

NCORES = 8
NGRP = 4             # branch groups
NBL = NB // NGRP     # 3 branches per core
LBH = B // 2         # 4096 rows per core (batch halves)
BT = 1024            # batch tile (pair of 512-column PSUM halves)
BH = 512             # PSUM half-tile
NBT = LBH // BT      # 4 batch tiles
M1 = D1 // 128       # 16 output tiles of layer 1
K2 = D1 // 128       # 16 contraction tiles of layer 2
M2 = D2 // 128       # 8
K3 = D2 // 128       # 8
M3 = D3 // 128       # 4
KH = D3 // 128       # 4
NAP = 12             # head width padded even

_NC_CACHE = {}
LAST_RESULT = None


def _build_nc():
    if "nc" in _NC_CACHE:
        return _NC_CACHE["nc"]
    from concourse import bacc
    import concourse.mybir as mybir
    import concourse.tile as tile

    f32 = mybir.dt.float32
    bf16 = mybir.dt.bfloat16
    Relu = mybir.ActivationFunctionType.Relu
    ADD = mybir.AluOpType.add
    MAX = mybir.AluOpType.max

    nc = bacc.Bacc("TRN2")

    px_d = nc.declare_dram_parameter("pxp", [NBL, D0P, LBH], bf16, isOutput=False)
    W1_d = nc.declare_dram_parameter("W1p", [NBL, D0P, D1], bf16, isOutput=False)
    W2_d = nc.declare_dram_parameter("W2p", [NBL, 128, K2, D2], bf16, isOutput=False)
    W3_d = nc.declare_dram_parameter("W3p", [NBL, 128, K3, D3], bf16, isOutput=False)
    Wq_d = nc.declare_dram_parameter("Wqp", [NBL, 128, KH, NAP], bf16, isOutput=False)
    b_d = nc.declare_dram_parameter("bp", [NBL, 128, M1 + M2 + M3], f32, isOutput=False)
    bq_d = nc.declare_dram_parameter("bqp", [NBL, NAP, 1], f32, isOutput=False)
    out_d = nc.declare_dram_parameter("out", [NBL, NAP, LBH], f32, isOutput=True)

    with tile.TileContext(nc) as tc:
        with (
            tc.tile_pool(name="wp1", bufs=2) as wp1,
            tc.tile_pool(name="wp2", bufs=2) as wp2,
            tc.tile_pool(name="wp3", bufs=2) as wp3,
            tc.tile_pool(name="wpq", bufs=2) as wpq,
            tc.tile_pool(name="bbp", bufs=2) as bbp,
            tc.tile_pool(name="pxp_s", bufs=2) as pxp,
            tc.tile_pool(name="h1p", bufs=2) as h1p,
            tc.tile_pool(name="actp", bufs=1) as actp,
            tc.tile_pool(name="osp", bufs=2) as osp,
            tc.tile_pool(name="psA", bufs=4, space="PSUM") as psA,
            tc.tile_pool(name="psS", bufs=4, space="PSUM") as psS,
        ):
            h2 = actp.tile([128, K3, BT], bf16, tag="h2")
            h3 = actp.tile([128, KH, BT], bf16, tag="h3")

            iters = [(br, bt) for br in range(NBL) for bt in range(NBT)]
            loaded = {}
            pxs = {}
            h1s = {}

            def load_branch(br):
                first = br == 0
                s_eng = nc.scalar if first else nc.sync
                w1t = wp1.tile([D0P, D1], bf16, tag="w1", name=f"w1_{br}")
                if first:
                    # branch 0 gates pipeline start: chunk W1/W2 on the sync
                    # ring so slices land just ahead of their use. Keep the
                    # scalar queue free — its engine must run the L1 drains.
                    for c in range(4):
                        nc.sync.dma_start(w1t[:, 512 * c:512 * (c + 1)],
                                          W1_d[br][:, 512 * c:512 * (c + 1)])
                else:
                    nc.sync.dma_start(w1t[:], W1_d[br])
                wqt = wpq.tile([128, KH, NAP], bf16, tag="wq", name=f"wq_{br}")
                s_eng.dma_start(wqt[:], Wq_d[br])
                btile = bbp.tile([128, M1 + M2 + M3], f32, tag="b", name=f"b_{br}")
                s_eng.dma_start(btile[:], b_d[br])
                bqt = bbp.tile([NAP, 1], f32, tag="bq", name=f"bq_{br}")
                s_eng.dma_start(bqt[:], bq_d[br])
                w2t = wp2.tile([128, K2, D2], bf16, tag="w2", name=f"w2_{br}")
                nch = 8 if first else 4
                kc = K2 // nch
                for c in range(nch):
                    nc.sync.dma_start(
                        w2t[:, kc * c:kc * (c + 1), :],
                        W2_d[br][:, kc * c:kc * (c + 1), :],
                    )
                w3t = wp3.tile([128, K3, D3], bf16, tag="w3", name=f"w3_{br}")
                nc.sync.dma_start(w3t[:], W3_d[br])
                loaded[br] = (w1t, w2t, w3t, wqt, btile, bqt)

            def load_px(idx):
                br, bt = iters[idx]
                bsl = slice(bt * BT, (bt + 1) * BT)
                px = pxp.tile([D0P, BT], bf16, tag="px", name=f"px_{idx}")
                nc.scalar.dma_start(px[:], px_d[br][:, bsl])
                pxs[idx] = px
                h1s[idx] = h1p.tile([128, M1, BT], bf16, tag="h1", name=f"h1_{idx}")

            def drain_split(dst, ps, bias, j):
                # whole-tile drains, alternating engines: half the PSUM read
                # transactions of a split drain — the PE PSUM write port runs
                # at ~99% occupancy, so every arbitration event costs a slot
                if j % 2 == 0:
                    nc.scalar.activation(dst, ps, Relu, bias=bias, scale=1.0)
                else:
                    nc.vector.tensor_scalar(dst, ps, bias, 0.0, ADD, MAX)

            def emit_L1_pair(idx, m):
                # one stationary, two N=512 matmuls (batch halves)
                br, _ = iters[idx]
                w1t, _, _, _, btile, _ = loaded[br]
                for h in range(2):
                    ps = psS.tile([128, BH], f32, tag="ps",
                                  name=f"l1ps_{idx}_{m}_{h}")
                    mm = nc.tensor.matmul(
                        ps[:], w1t[:, m * 128:(m + 1) * 128],
                        pxs[idx][:, h * BH:(h + 1) * BH],
                        start=True, stop=True,
                    )
                    if h == 1:
                        mm.ins.ldweights = False  # stationary already loaded
                    drain_split(h1s[idx][:, m, h * BH:(h + 1) * BH], ps[:],
                                btile[:, m:m + 1], m + h)

            def head_state(idx):
                psh = [psS.tile([NAP, BH], f32, tag="ps",
                                name=f"psh_{idx}_{h}") for h in range(2)]
                return (idx, psh)

            def head_mms(state, k):
                idx, psh = state
                br, _ = iters[idx]
                wqt = loaded[br][3]
                for h in range(2):
                    mm = nc.tensor.matmul(
                        psh[h][:], wqt[:, k, :],
                        h3[:, k, h * BH:(h + 1) * BH],
                        start=(k == 0), stop=(k == KH - 1),
                    )
                    if h == 1:
                        mm.ins.ldweights = False

            def head_finish(state, out_eng=None):
                idx, psh = state
                br, bt = iters[idx]
                bqt = loaded[br][5]
                ot = osp.tile([NAP, BT], f32, tag="os", name=f"ot_{idx}")
                for h in range(2):
                    nc.vector.tensor_scalar_add(
                        ot[:, h * BH:(h + 1) * BH], psh[h][:], bqt[:])
                bsl = slice(bt * BT, (bt + 1) * BT)
                (out_eng or nc.gpsimd).dma_start(out_d[br][:, bsl], ot[:])

            def emit_head(idx, out_eng=None):
                st = head_state(idx)
                for k in range(KH):
                    head_mms(st, k)
                head_finish(st, out_eng)

            # prologue: px + W1 land first; while they are in flight, run
            # throwaway matmuls on a zeroed tile — they overlap the DMA wait
            # and push the PE HAM clock-gate to full rate (2.4 GHz) before
            # real work arrives
            load_px(0)
            load_branch(0)
            warm = actp.tile([128, BH], bf16, tag="warm")
            nc.vector.memset(warm[:], 0)
            for w in range(10):
                ps = psS.tile([128, BH], f32, tag="ps", name=f"warm_{w}")
                nc.tensor.matmul(ps[:], warm[:, 0:128], warm[:],
                                 start=True, stop=True)
            for m in range(M1):
                emit_L1_pair(0, m)

            for idx, (br, bt) in enumerate(iters):
                w1t, w2t, w3t, wqt, btile, bqt = loaded[br]
                h1 = h1s[idx]
                nxt = idx + 1
                have_nxt = nxt < len(iters)
                if have_nxt:
                    load_px(nxt)
                if bt == 0 and br + 1 < NBL:
                    load_branch(br + 1)

                fill = iter(range(M1))  # next-iteration L1 m indices

                def fillers(n):
                    if have_nxt:
                        for _ in range(n):
                            m = next(fill, None)
                            if m is not None:
                                emit_L1_pair(nxt, m)

                last = idx == len(iters) - 1
                # prev head at iteration top covers the ps3(idx-1) -> L2 g0
                # PSUM drain latency; in the last iteration (which has no L1
                # fillers) it is spread across the L2 group boundaries instead
                hd_prev = None
                if idx > 0:
                    if last:
                        hd_prev = head_state(idx - 1)
                        head_mms(hd_prev, 0)
                    else:
                        emit_head(idx - 1)

                # ---- L2 [2048 -> 1024]: 4 m-groups of 2, k-outer,
                #      each stationary feeds both batch halves ----
                for g in range(4):
                    ps2 = [psA.tile([128, BH], f32, tag="ps",
                                    name=f"ps2_{idx}_{g}_{j}") for j in range(4)]
                    for k in range(K2):
                        for jm in range(2):
                            m = 2 * g + jm
                            for h in range(2):
                                mm = nc.tensor.matmul(
                                    ps2[2 * jm + h][:],
                                    w2t[:, k, m * 128:(m + 1) * 128],
                                    h1[:, k, h * BH:(h + 1) * BH],
                                    start=(k == 0), stop=(k == K2 - 1),
                                )
                                if h == 1:
                                    mm.ins.ldweights = False
                    for jm in range(2):
                        m = 2 * g + jm
                        for h in range(2):
                            drain_split(h2[:, m, h * BH:(h + 1) * BH],
                                        ps2[2 * jm + h][:],
                                        btile[:, M1 + m:M1 + m + 1], jm + h)
                    fillers(2)  # hide this group's drains
                    if hd_prev is not None and g < KH - 1:
                        head_mms(hd_prev, g + 1)
                    if hd_prev is not None and g == KH - 1:
                        head_finish(hd_prev)

                # ---- L3 [1024 -> 512]: 2 m-groups of 2, k-outer ----
                for g in range(2):
                    ps3 = [psA.tile([128, BH], f32, tag="ps",
                                    name=f"ps3_{idx}_{g}_{j}") for j in range(4)]
                    for k in range(K3):
                        for jm in range(2):
                            m = 2 * g + jm
                            for h in range(2):
                                mm = nc.tensor.matmul(
                                    ps3[2 * jm + h][:],
                                    w3t[:, k, m * 128:(m + 1) * 128],
                                    h2[:, k, h * BH:(h + 1) * BH],
                                    start=(k == 0), stop=(k == K3 - 1),
                                )
                                if h == 1:
                                    mm.ins.ldweights = False
                        if k == 3:
                            fillers(2)  # spread remaining L1 work
                    for jm in range(2):
                        m = 2 * g + jm
                        for h in range(2):
                            drain_split(h3[:, m, h * BH:(h + 1) * BH],
                                        ps3[2 * jm + h][:],
                                        btile[:, M1 + M2 + m:M1 + M2 + m + 1],
                                        jm + h)
                    fillers(2)
                    if last:
                        if g == 0:
                            hd_last = head_state(idx)
                            head_mms(hd_last, 0)
                            head_mms(hd_last, 1)
                        else:
                            head_mms(hd_last, 2)
                            head_mms(hd_last, 3)
                            # final out through the scalar HWDGE queue: the
                            # gpsimd SWDGE latency would sit exposed at the
                            # kernel tail
                            head_finish(hd_last, out_eng=nc.scalar)
                fillers(M1)  # flush any remaining next-L1 pairs

    nc.compile()
    _NC_CACHE["nc"] = nc
    return nc


def _pack_weights(W1, b1, W2, b2, W3, b3, Wv, bv, Wa, ba):
    f = np.float32
    W1p = np.zeros((NB, D0P, D1), BF16)                           # [12, 128, 2048]
    W1p[:, :D0, :] = np.ascontiguousarray(W1).astype(BF16)
    W2p = np.ascontiguousarray(
        W2.reshape(NB, K2, 128, D2).transpose(0, 2, 1, 3)).astype(BF16)
    W3p = np.ascontiguousarray(
        W3.reshape(NB, K3, 128, D3).transpose(0, 2, 1, 3)).astype(BF16)
    # fold dueling head: q = h @ (Wv + Wa - mean(Wa)) + (bv + ba - mean(ba))
    Wq = Wv + Wa - Wa.mean(axis=2, keepdims=True)                 # [12, 512, 11]
    bq = bv + ba - ba.mean(axis=1, keepdims=True)                 # [12, 11]
    Wq = np.concatenate([Wq, np.zeros((NB, D3, NAP - NA), Wq.dtype)], axis=2)
    bq = np.concatenate([bq, np.zeros((NB, NAP - NA), bq.dtype)], axis=1)
    Wqp = np.ascontiguousarray(
        Wq.reshape(NB, KH, 128, NAP).transpose(0, 2, 1, 3)).astype(BF16)
    bp = np.concatenate(
        [
            b1.reshape(NB, M1, 128).transpose(0, 2, 1),
            b2.reshape(NB, M2, 128).transpose(0, 2, 1),
            b3.reshape(NB, M3, 128).transpose(0, 2, 1),
        ],
        axis=2,
    ).astype(f)                                                   # [12, 128, 28]
    bqp = np.ascontiguousarray(bq[:, :, None], f)                 # [12, 12, 1]
    return W1p, W2p, W3p, Wqp, bp, bqp


def kernel(x, W1, b1, W2, b2, W3, b3, Wv, bv, Wa, ba):
    global LAST_RESULT
    from concourse.bass_utils import run_bass_kernel_spmd

    x = np.asarray(x, np.float32)
    args = [np.asarray(a, np.float32) for a in (W1, b1, W2, b2, W3, b3, Wv, bv, Wa, ba)]
    W1p, W2p, W3p, Wqp, bp, bqp = _pack_weights(*args)

    node = x[:, :NODE]                                  # [B, 45]
    groups = x[:, NODE:].reshape(B, NB, GRP)            # [B, 12, 17]

    nc = _build_nc()
    in_maps = []
    for c in range(NCORES):
        g, h = c // 2, c % 2
        rows = slice(h * LBH, (h + 1) * LBH)
        brs = slice(g * NBL, (g + 1) * NBL)
        pxc = np.zeros((NBL, D0P, LBH), BF16)
        nT = np.ascontiguousarray(node[rows].T).astype(BF16)
        for j in range(NBL):
            pxc[j, :NODE] = nT
            pxc[j, NODE:D0] = groups[rows, g * NBL + j].T
        in_maps.append({
            "pxp": pxc,
            "W1p": W1p[brs], "W2p": W2p[brs], "W3p": W3p[brs], "Wqp": Wqp[brs],
            "bp": bp[brs], "bqp": bqp[brs],
        })

    res = run_bass_kernel_spmd(nc, in_maps, list(range(NCORES)))
    LAST_RESULT = res

    out = np.empty((NB, B, NA), np.float32)
    for c in range(NCORES):
        g, h = c // 2, c % 2
        rows = slice(h * LBH, (h + 1) * LBH)
        oc = res.results[c]["out"]                      # [3, 12, 4096]
        for j in range(NBL):
            out[g * NBL + j, rows, :] = oc[j, :NA, :].T
    return out
